# revision 30
# baseline (speedup 1.0000x reference)
"""Trainium2 Bass kernel for nn_AWGNIndexChannelWrapper.

Reference computation:
  rx_c = bitflip(idx_c, flip_u_c)  (9-bit symbols, per-bit XOR with (u < BER))
  rx_f = bitflip(idx_f, flip_u_f)
  out  = concat([codebook_f[rx_f].reshape(B, -1), codebook_c[rx_c].reshape(B, -1)], axis=1)

Key simplification: packing/unpacking 9-bit symbols with per-bit XOR is just
  rx = idx ^ flipmask,  flipmask = sum_k (u_k < BER) << k
and the clip is a no-op (9-bit values are already < 512).

Sharding: pure data parallel over the batch dim (64 batches -> 8 cores x 8).

Per core:
  1. load idx/flip slices into SBUF (natural [128, N/128] layout),
  2. compute rx = idx ^ flipmask on the vector engine, convert to int16,
  3. move rx through a DRAM scratch round-trip into the wrapped index layout
     dma_gather requires ([16, n/16] with index i at (i%16, i//16), replicated
     across all 8 partition groups -- the GPSIMD cores read hardwired groups),
  4. per batch, dma_gather 512B codebook rows from HBM into SBUF,
  5. write gathered tiles to the output with a 3-dim strided DMA that lands
     them exactly in the reference's row-major layout.

The coarse region (1/4 the work) is processed first so its gathers feed the
DMA engines while the fine region's compute/scratch prologue is still running.

Gather call b uses indices rx[16b+q, sigma(t)] at wrapped position (q, t),
with sigma(t) = (t%8)*(F/8) + t//8 folded into the rx int16 copy as a free AP
transpose. Unwrapping (idx i at partition i%16, slot i//16) plus the dst rule
(position g lands at [g%128, g//128]) then puts within-batch point
m = (P%16)*F + (P//16)*(F/8) + S' at dst [P, S'] -- consecutive S' are
consecutive points, so each partition's S*512B land as one contiguous DRAM
run and the write AP is [ph: (F/8)*128elem x8][pl: F*128elem x16][16KB contig].
"""

import os

import numpy as np

import concourse.bacc as bacc
import concourse.mybir as mybir
import concourse.tile as tile
from concourse.bass_utils import run_bass_kernel_spmd

# Problem constants (hardcoded per harness contract).
BER = 0.02
BITS = 9
KC = KF = 512
B, HC, WC, HF, WF, D = 64, 32, 32, 64, 64, 128

N_CORES = 8
B_LOC = B // N_CORES          # 8 batches per core
NF = B_LOC * HF * WF          # 32768 fine points per core
NC_ = B_LOC * HC * WC         # 8192 coarse points per core
P = 128
FF = NF // P                  # 256 fine slots per partition
FC = NC_ // P                 # 64 coarse slots per partition
FINE_ROW = HF * WF * D        # 524288 f32 per output row (fine region)
COARSE_ROW = HC * WC * D      # 131072 f32 per output row (coarse region)
OUT_ROW = FINE_ROW + COARSE_ROW

f32 = mybir.dt.float32
i32 = mybir.dt.int32
i16 = mybir.dt.int16


def _compute_rx(nc, pool, idx_dram, flip_dram, n_pts, rx16_out, tag):
    """rx16_out[p, f] = (idx ^ flipmask)[p * (n_pts/128) + f] as int16."""
    F = n_pts // P
    idx_t = pool.tile([P, F], i32, tag=f"idx{tag}")
    nc.sync.dma_start(
        idx_t[:],
        idx_dram.ap().rearrange("b h w -> (b h w)").rearrange("(p f) -> p f", p=P),
    )
    u_t = pool.tile([P, F * BITS], f32, tag=f"u{tag}")
    nc.sync.dma_start(
        u_t[:],
        flip_dram.ap()
        .rearrange("b h w k -> (b h w) k")
        .rearrange("(p f) k -> p (f k)", p=P),
    )

    # sc[p, f, k] = (u[p, f, k] < BER) * 2^k, one fused op per bit plane
    sc_t = pool.tile([P, F * BITS], f32, tag=f"sc{tag}")
    u_v = u_t[:].rearrange("p (f k) -> p f k", k=BITS)
    sc_v = sc_t[:].rearrange("p (f k) -> p f k", k=BITS)
    for k in range(BITS):
        nc.vector.tensor_scalar(
            out=sc_v[:, :, k],
            in0=u_v[:, :, k],
            scalar1=BER,
            scalar2=float(1 << k),
            op0=mybir.AluOpType.is_lt,
            op1=mybir.AluOpType.mult,
        )
    fm_t = pool.tile([P, F], f32, tag=f"fm{tag}")
    nc.vector.tensor_reduce(
        out=fm_t[:],
        in_=sc_t[:].rearrange("p (f k) -> p f k", k=BITS),
        axis=mybir.AxisListType.X,
        op=mybir.AluOpType.add,
    )
    fm_i = pool.tile([P, F], i32, tag=f"fmi{tag}")
    nc.vector.tensor_copy(out=fm_i[:], in_=fm_t[:])
    rx_t = pool.tile([P, F], i32, tag=f"rx{tag}")
    nc.vector.tensor_tensor(
        out=rx_t[:], in0=idx_t[:], in1=fm_i[:], op=mybir.AluOpType.bitwise_xor
    )
    # sigma-permute the per-partition order (t = s*8+h reads rx[h*(F/8)+s])
    # so each dst partition's points land consecutively in DRAM (16KB runs).
    nc.vector.tensor_copy(out=rx16_out, in_=rx_t[:].rearrange("p (h s) -> p s h", h=8))


def _region(nc, io_pool, g_pool, dram_pool, out, *, idx_dram, flip_dram, cb,
            n_pts, row_slice, tag, n_queues, call_counter):
    """Full pipeline for one region (coarse or fine)."""
    F = n_pts // P
    S = 16 * F // P           # dst slots per partition per batch call

    rx16 = io_pool.tile([P, F], i16, tag=f"rx16{tag}")
    _compute_rx(nc, io_pool, idx_dram, flip_dram, n_pts, rx16[:], tag)

    # DRAM round trip to the wrapped, group-replicated index tile W:
    #   W[16h+q, b*F + t] = rx[16b+q, t]  for all h
    # d1 is [q][b][t] so the reload is one contiguous run per partition and
    # the x8 group replication is a single zero-stride broadcast DMA.
    d1 = dram_pool.tile([16, B_LOC, F], i16, tag=f"d1{tag}")
    nc.scalar.dma_start(d1[:].transpose([1, 0, 2]), rx16[:])

    W = io_pool.tile([P, B_LOC * F], i16, tag=f"W{tag}")
    nc.scalar.dma_start(
        W[:],
        d1[:]
        .rearrange("q b t -> q (b t)")
        .unsqueeze(0)
        .broadcast_to([8, 16, B_LOC * F]),
    )

    # Call tiling: full batches early; split the trailing batches into
    # smaller calls so the final DMA work (the kernel tail) is short.
    # With the sigma layout any S'-subrange keeps a 3-dim write AP.
    for b in range(B_LOC):
        if tag == "f" and b == B_LOC - 2:
            splits = [S // 2, S // 2]
        elif tag == "f" and b == B_LOC - 1:
            splits = [S // 4] * 4
        else:
            splits = [S]
        view = out.ap()[b, row_slice].rearrange(
            "(pl ph s d) -> ph pl (s d)", pl=16, s=S, ph=8, d=D
        )
        s_off = 0
        for j, s_sub in enumerate(splits):
            n_idx = s_sub * P
            c0 = b * F + s_off * 8
            gt = g_pool.tile([P, s_sub, D], f32, tag=f"g{tag}")
            nc.gpsimd.dma_gather(
                gt[:],
                cb.ap(),
                W[:, c0 : c0 + n_idx // 16],
                n_idx,
                n_idx,
                D,
                queue_num=call_counter[0] % n_queues,
                single_packet=False,
            )
            call_counter[0] += 1
            nc.sync.dma_start(
                out=view[:, :, s_off * D : (s_off + s_sub) * D], in_=gt[:]
            )
            s_off += s_sub


def build_nc():
    n_queues = int(os.environ.get("K_NQ", "4"))
    nc = bacc.Bacc(
        "TRN2", target_bir_lowering=False, debug=False, num_swdge_queues=n_queues,
        dynamic_dma_scratch_size=int(os.environ.get("K_RING", "65536")),
    )

    idx_c = nc.dram_tensor("idx_c", [B_LOC, HC, WC], i32, kind="ExternalInput")
    idx_f = nc.dram_tensor("idx_f", [B_LOC, HF, WF], i32, kind="ExternalInput")
    cb_c = nc.dram_tensor("codebook_c", [KC, D], f32, kind="ExternalInput")
    cb_f = nc.dram_tensor("codebook_f", [KF, D], f32, kind="ExternalInput")
    fu_c = nc.dram_tensor("flip_u_c", [B_LOC, HC, WC, BITS], f32, kind="ExternalInput")
    fu_f = nc.dram_tensor("flip_u_f", [B_LOC, HF, WF, BITS], f32, kind="ExternalInput")
    out = nc.dram_tensor("out", [B_LOC, OUT_ROW], f32, kind="ExternalOutput")

    with tile.TileContext(nc) as tc:
        with (
            tc.tile_pool(name="io", bufs=1) as io_pool,
            tc.tile_pool(name="g", bufs=int(os.environ.get("K_BUFS", "4"))) as g_pool,
            tc.tile_pool(name="dram", bufs=1, space="DRAM") as dram_pool,
        ):
            call_counter = [0]
            for _rep in range(int(os.environ.get("K_REPS", "1"))):
                # Coarse first: its gathers feed the DMA engines while the
                # fine region's compute/scratch prologue runs.
                _region(
                    nc, io_pool, g_pool, dram_pool, out,
                    idx_dram=idx_c, flip_dram=fu_c, cb=cb_c, n_pts=NC_,
                    row_slice=slice(FINE_ROW, OUT_ROW), tag="c",
                    n_queues=n_queues, call_counter=call_counter,
                )
                _region(
                    nc, io_pool, g_pool, dram_pool, out,
                    idx_dram=idx_f, flip_dram=fu_f, cb=cb_f, n_pts=NF,
                    row_slice=slice(0, FINE_ROW), tag="f",
                    n_queues=n_queues, call_counter=call_counter,
                )

    nc.compile()
    return nc


_NC_CACHE = None


def _get_nc():
    global _NC_CACHE
    if _NC_CACHE is None:
        _NC_CACHE = build_nc()
    return _NC_CACHE


def _in_maps(idx_c, idx_f, codebook_c, codebook_f, flip_u_c, flip_u_f):
    maps = []
    for c in range(N_CORES):
        b0, b1 = c * B_LOC, (c + 1) * B_LOC
        maps.append(
            {
                "idx_c": np.ascontiguousarray(idx_c[b0:b1]),
                "idx_f": np.ascontiguousarray(idx_f[b0:b1]),
                "codebook_c": np.ascontiguousarray(codebook_c),
                "codebook_f": np.ascontiguousarray(codebook_f),
                "flip_u_c": np.ascontiguousarray(flip_u_c[b0:b1]),
                "flip_u_f": np.ascontiguousarray(flip_u_f[b0:b1]),
            }
        )
    return maps


class _AxonRunner:
    """Cached sharded PJRT executable for the axon path.

    run_bass_kernel_spmd rebuilds its jit closure (and retraces) on every
    call; caching the executable makes repeat kernel() calls cheap. Uses the
    same bass2jax machinery run_bass_kernel_spmd itself uses under axon.
    """

    def __init__(self, nc):
        import jax
        from jax.sharding import Mesh, NamedSharding, PartitionSpec
        from jax.experimental.shard_map import shard_map
        import concourse.bass2jax as b2j

        b2j.install_neuronx_cc_hook()
        self._jax = jax
        pname = nc.partition_id_tensor.name if nc.partition_id_tensor else None
        in_names, out_names, out_avals, zeros = [], [], [], []
        for alloc in nc.m.functions[0].allocations:
            if not isinstance(alloc, mybir.MemoryLocationSet):
                continue
            name = alloc.memorylocations[0].name
            if alloc.kind == "ExternalInput":
                if name != pname:
                    in_names.append(name)
            elif alloc.kind == "ExternalOutput":
                out_names.append(name)
                shape = tuple(alloc.tensor_shape)
                dtype = mybir.dt.np(alloc.dtype)
                out_avals.append(jax.core.ShapedArray(shape, dtype))
                zeros.append(np.zeros((N_CORES * shape[0], *shape[1:]), dtype))
        self.in_names = in_names
        all_in = in_names + out_names + ([pname] if pname else [])

        def _body(*args):
            ops = list(args)
            if pname is not None:
                ops.append(b2j.partition_id_tensor())
            return tuple(
                b2j._bass_exec_p.bind(
                    *ops,
                    out_avals=tuple(out_avals),
                    in_names=tuple(all_in),
                    out_names=tuple(out_names),
                    lowering_input_output_aliases=(),
                    sim_require_finite=True,
                    sim_require_nnan=True,
                    nc=nc,
                )
            )

        devices = jax.devices()[:N_CORES]
        mesh = Mesh(np.asarray(devices), ("core",))
        n = len(in_names) + len(out_names)
        self.sharded = jax.jit(
            shard_map(
                _body,
                mesh=mesh,
                in_specs=(PartitionSpec("core"),) * n,
                out_specs=(PartitionSpec("core"),) * len(out_names),
                check_rep=False,
            ),
            keep_unused=True,
        )
        self.sh = NamedSharding(mesh, PartitionSpec("core"))
        self.dev_zeros = [jax.device_put(z, self.sh) for z in zeros]

    def run(self, full):
        jax = self._jax
        dev_in = [jax.device_put(full[n], self.sh) for n in self.in_names]
        outs = self.sharded(*dev_in, *self.dev_zeros)
        return np.asarray(outs[0]).reshape(B, OUT_ROW)


_RUNNER = None


def kernel(idx_c, idx_f, codebook_c, codebook_f, flip_u_c, flip_u_f):
    from concourse._compat import axon_active

    if axon_active():
        global _RUNNER
        if _RUNNER is None:
            _RUNNER = _AxonRunner(_get_nc())
        full = {
            "idx_c": np.ascontiguousarray(idx_c),
            "idx_f": np.ascontiguousarray(idx_f),
            "codebook_c": np.tile(np.ascontiguousarray(codebook_c), (N_CORES, 1)),
            "codebook_f": np.tile(np.ascontiguousarray(codebook_f), (N_CORES, 1)),
            "flip_u_c": np.ascontiguousarray(flip_u_c),
            "flip_u_f": np.ascontiguousarray(flip_u_f),
        }
        return _RUNNER.run(full)

    nc = _get_nc()
    maps = _in_maps(idx_c, idx_f, codebook_c, codebook_f, flip_u_c, flip_u_f)
    res = run_bass_kernel_spmd(nc, maps, core_ids=list(range(N_CORES)))
    return np.concatenate([r["out"] for r in res.results], axis=0)
